# revision 2
# baseline (speedup 1.0000x reference)
"""Trainium2 Bass kernel for batched additive-attention scoring.

Computes, for each batch b:
    out[b] = softmax_s( sum_h v[h] * tanh( (W1 @ static[b])[h,s]
                                         + (W2 @ dynamic[b])[h,s]
                                         + (W3 @ hidden[b])[h] ) )

Sharding: data-parallel over batch B=64 across 8 NeuronCores (8 batches
per core); small params (W, v) replicated.  No collectives needed.

Per-core dataflow (H=256, S=4096):
  - wt  [512,256]  = [W1^T ; W2^T] host-pretransposed, k-major
  - per batch: DMA 4 k-chunks [128,4096] (2 static + 2 dynamic)
  - per s-tile (512 cols), per m-block (128 rows of h):
      PSUM  E = sum_k wt_chunk^T @ x_chunk      (4 f32r matmuls, K=512 total)
      SBUF  Eb = tanh(E + bias[h,b])            (ACT, per-partition bias)
  - scores[1,512] = v0^T @ Eb0 + v1^T @ Eb1     (2 f32r matmuls, M=1)
  - exp row + running sum via ACT Exp accum_out; final 1/sum scale on DVE
"""

import os
import sys
from contextlib import ExitStack

import numpy as np

for _p in ("/root/.axon_site", "/root/.axon_site/_ro/trn_rl_repo",
           "/root/.axon_site/_ro/pypackages", "/opt/trn_rl_repo", "/opt/pypackages"):
    if os.path.isdir(_p) and _p not in sys.path:
        sys.path.append(_p)

import concourse.bass as bass
import concourse.tile as tile
from concourse import bacc, mybir
from concourse._compat import with_exitstack
from concourse.bass_utils import run_bass_kernel_spmd

H = 256
S = 4096
B = 64
NCORES = 8
BPC = B // NCORES  # batches per core

F32 = mybir.dt.float32
F32R = mybir.dt.float32r
TANH = mybir.ActivationFunctionType.Tanh
EXP = mybir.ActivationFunctionType.Exp

ST = 512           # s-tile width (one PSUM bank of f32)
NS = S // ST       # 8 s-tiles
NM = H // 128      # 2 m-blocks (output h partition blocks)
NK = (2 * H) // 128  # 4 k-chunks (static 0..1, dynamic 2..3)


@with_exitstack
def _attn_kernel(ctx: ExitStack, tc: "tile.TileContext",
                 out_ap, static_ap, dyn_ap, wt_ap, w3t_ap, vt_ap, ht_ap):
    nc = tc.nc

    const = ctx.enter_context(tc.tile_pool(name="const", bufs=1))
    xpool = ctx.enter_context(tc.tile_pool(name="x", bufs=6))
    epsum = ctx.enter_context(tc.tile_pool(name="epsum", bufs=4, space="PSUM"))
    spsum = ctx.enter_context(tc.tile_pool(name="spsum", bufs=2, space="PSUM"))
    bpsum = ctx.enter_context(tc.tile_pool(name="bpsum", bufs=1, space="PSUM"))
    esb = ctx.enter_context(tc.tile_pool(name="esb", bufs=4))
    rows = ctx.enter_context(tc.tile_pool(name="rows", bufs=2))
    tiny = ctx.enter_context(tc.tile_pool(name="tiny", bufs=4))

    # ---- load replicated params (host already laid out partition-major) ----
    wt_sb = const.tile([128, NK, H], F32R)      # [p, kchunk, h]
    nc.sync.dma_start(wt_sb[:], wt_ap)
    w3_sb = const.tile([128, 2, H], F32R)       # [p, kchunk, h]
    nc.sync.dma_start(w3_sb[:], w3t_ap)
    vt_sb = const.tile([128, 2], F32R)          # [p, hchunk]
    nc.sync.dma_start(vt_sb[:], vt_ap)
    ht_sb = const.tile([128, 2, BPC], F32R)     # [p, kchunk, b]
    nc.sync.dma_start(ht_sb[:], ht_ap)

    # ---- bias[h, b] = sum_k W3T[k,h] * hiddenT[k,b] (all batches at once) ----
    bias_sb = const.tile([128, NM, BPC], F32)  # [p, m, b]
    for m in range(NM):
        bp = bpsum.tile([128, BPC], F32)
        for c in range(2):
            nc.tensor.matmul(bp[:],
                             lhsT=w3_sb[:, c, m * 128:(m + 1) * 128],
                             rhs=ht_sb[:, c, :],
                             start=(c == 0), stop=(c == 1))
        nc.vector.tensor_copy(bias_sb[:, m, :], bp[:])

    for b in range(BPC):
        # stream this batch's data: 4 k-chunks of [128, S]
        xs = []
        for src, base in ((static_ap, 0), (dyn_ap, 2)):
            for c in range(2):
                xt = xpool.tile([128, S], F32R, tag="x")
                nc.sync.dma_start(xt[:], src[b, c * 128:(c + 1) * 128, :])
                xs.append(xt)

        exp_row = rows.tile([1, S], F32, tag="exp")
        sums = tiny.tile([1, NS], F32, tag="sums")

        for s in range(NS):
            es_tiles = []
            for m in range(NM):
                ep = epsum.tile([128, ST], F32)
                for c in range(NK):
                    nc.tensor.matmul(ep[:],
                                     lhsT=wt_sb[:, c, m * 128:(m + 1) * 128],
                                     rhs=xs[c][:, s * ST:(s + 1) * ST],
                                     start=(c == 0), stop=(c == NK - 1))
                es = esb.tile([128, ST], F32R)
                nc.scalar.activation(es[:], ep[:], TANH,
                                     bias=bias_sb[:, m, b:b + 1])
                es_tiles.append(es)

            sp = spsum.tile([1, ST], F32)
            nc.tensor.matmul(sp[:], lhsT=vt_sb[:, 0:1], rhs=es_tiles[0][:],
                             start=True, stop=False)
            nc.tensor.matmul(sp[:], lhsT=vt_sb[:, 1:2], rhs=es_tiles[1][:],
                             start=False, stop=True)

            nc.scalar.activation(exp_row[:, s * ST:(s + 1) * ST], sp[:], EXP,
                                 accum_out=sums[:, s:s + 1])

        tot = tiny.tile([1, 1], F32, tag="tot")
        nc.vector.tensor_reduce(tot[:], sums[:], axis=mybir.AxisListType.X,
                                op=mybir.AluOpType.add)
        inv = tiny.tile([1, 1], F32, tag="inv")
        nc.vector.reciprocal(inv[:], tot[:])
        fin = rows.tile([1, S], F32, tag="fin")
        nc.vector.tensor_scalar_mul(fin[:], exp_row[:], inv[:, 0:1])
        nc.sync.dma_start(out_ap[b:b + 1, :], fin[:])


_CACHED = None


def _build():
    global _CACHED
    if _CACHED is not None:
        return _CACHED
    nc = bacc.Bacc("TRN2", target_bir_lowering=False, debug=False,
                   num_devices=NCORES)
    static = nc.dram_tensor("static", (BPC, H, S), F32R, kind="ExternalInput").ap()
    dyn = nc.dram_tensor("dynamic", (BPC, H, S), F32R, kind="ExternalInput").ap()
    wt = nc.dram_tensor("wt", (128, NK, H), F32R, kind="ExternalInput").ap()
    w3t = nc.dram_tensor("w3t", (128, 2, H), F32R, kind="ExternalInput").ap()
    vt = nc.dram_tensor("vt", (128, 2), F32R, kind="ExternalInput").ap()
    ht = nc.dram_tensor("ht", (128, 2, BPC), F32R, kind="ExternalInput").ap()
    out = nc.dram_tensor("out", (BPC, S), F32, kind="ExternalOutput").ap()

    with tile.TileContext(nc) as tc:
        _attn_kernel(tc, out, static, dyn, wt, w3t, vt, ht)
    nc.compile()
    _CACHED = nc
    return nc


def _chunk_major(a: np.ndarray) -> np.ndarray:
    """[C*128, F] -> [128, C, F] so partition p holds rows {p, 128+p, ...}."""
    c = a.shape[0] // 128
    return np.ascontiguousarray(a.reshape(c, 128, -1).transpose(1, 0, 2))


def kernel(static_enc, dynamic_enc, decoder_hidden, v, W, *, _trace=False,
           **trace_kwargs):
    static_enc = np.ascontiguousarray(static_enc, dtype=np.float32)
    dynamic_enc = np.ascontiguousarray(dynamic_enc, dtype=np.float32)
    decoder_hidden = np.ascontiguousarray(decoder_hidden, dtype=np.float32)
    v = np.ascontiguousarray(v, dtype=np.float32)
    W = np.ascontiguousarray(W, dtype=np.float32)

    nc = _build()

    wt = _chunk_major(np.concatenate([W[:, :H].T, W[:, H:2 * H].T], axis=0))
    w3t = _chunk_major(np.ascontiguousarray(W[:, 2 * H:].T))
    vt = np.ascontiguousarray(v.reshape(2, 128).T)          # [128, 2]
    in_maps = []
    for i in range(NCORES):
        sl = slice(i * BPC, (i + 1) * BPC)
        ht = _chunk_major(np.ascontiguousarray(decoder_hidden[sl].T))
        in_maps.append({
            "static": static_enc[sl],
            "dynamic": dynamic_enc[sl],
            "wt": wt, "w3t": w3t, "vt": vt, "ht": ht,
        })

    res = run_bass_kernel_spmd(nc, in_maps, core_ids=list(range(NCORES)),
                               trace=_trace, **trace_kwargs)
    kernel.last_result = res
    return np.concatenate([res.results[i]["out"] for i in range(NCORES)], axis=0)


kernel.last_result = None


# revision 7
# speedup vs baseline: 1.2850x; 1.2850x over previous
"""Trainium2 Bass kernel for batched additive-attention scoring.

Computes, for each batch b:
    out[b] = softmax_s( sum_h v[h] * tanh( (W1 @ static[b])[h,s]
                                         + (W2 @ dynamic[b])[h,s]
                                         + (W3 @ hidden[b])[h] ) )

Sharding: data-parallel over batch B=64 across 8 NeuronCores (8 batches
per core); small params (W, v) replicated.  No collectives needed.

Per-core dataflow (H=256, S=4096):
  - wt  [512,256]  = [W1^T ; W2^T] host-pretransposed, k-major
  - per batch: DMA 4 k-chunks [128,4096] (2 static + 2 dynamic)
  - per s-tile (512 cols), per m-block (128 rows of h):
      PSUM  E = sum_k wt_chunk^T @ x_chunk      (4 f32r matmuls, K=512 total)
      SBUF  Eb = tanh(E + bias[h,b])            (ACT, per-partition bias)
  - scores[1,512] = v0^T @ Eb0 + v1^T @ Eb1     (2 f32r matmuls, M=1)
  - exp row + running sum via ACT Exp accum_out; final 1/sum scale on DVE
"""

import os
import sys
from contextlib import ExitStack

import numpy as np

for _p in ("/root/.axon_site", "/root/.axon_site/_ro/trn_rl_repo",
           "/root/.axon_site/_ro/pypackages", "/opt/trn_rl_repo", "/opt/pypackages"):
    if os.path.isdir(_p) and _p not in sys.path:
        sys.path.append(_p)

import concourse.bass as bass
import concourse.tile as tile
from concourse import bacc, mybir
from concourse._compat import with_exitstack
from concourse.bass_utils import run_bass_kernel_spmd

H = 256
S = 4096
B = 64
NCORES = 8
BPC = B // NCORES  # batches per core

F32 = mybir.dt.float32
F32R = mybir.dt.float32r
TANH = mybir.ActivationFunctionType.Tanh
EXP = mybir.ActivationFunctionType.Exp

ST = 512           # s-tile width (one PSUM bank of f32)
NS = S // ST       # 8 s-tiles
NM = H // 128      # 2 m-blocks (output h partition blocks)
NK = (2 * H) // 128  # 4 k-chunks (static 0..1, dynamic 2..3)


@with_exitstack
def _attn_kernel(ctx: ExitStack, tc: "tile.TileContext",
                 out_ap, static_ap, dyn_ap, wt_ap, w3t_ap, vt_ap, ht_ap):
    nc = tc.nc

    const = ctx.enter_context(tc.tile_pool(name="const", bufs=1))
    xpool = ctx.enter_context(tc.tile_pool(name="x", bufs=8))
    epsum = ctx.enter_context(tc.tile_pool(name="epsum", bufs=4, space="PSUM"))
    spsum = ctx.enter_context(tc.tile_pool(name="spsum", bufs=2, space="PSUM"))
    bpsum = ctx.enter_context(tc.tile_pool(name="bpsum", bufs=1, space="PSUM"))
    esb = ctx.enter_context(tc.tile_pool(name="esb", bufs=6))
    rows = ctx.enter_context(tc.tile_pool(name="rows", bufs=2))
    tiny = ctx.enter_context(tc.tile_pool(name="tiny", bufs=4))

    # ---- load replicated params (host already laid out partition-major) ----
    wt_sb = const.tile([128, NK, H], F32R)      # [p, kchunk, h]
    nc.sync.dma_start(wt_sb[:], wt_ap)
    w3_sb = const.tile([128, 2, H], F32R)       # [p, kchunk, h]
    nc.sync.dma_start(w3_sb[:], w3t_ap)
    vt_sb = const.tile([128, 2], F32R)          # [p, hchunk]
    nc.sync.dma_start(vt_sb[:], vt_ap)
    ht_sb = const.tile([128, 2, BPC], F32R)     # [p, kchunk, b]
    nc.sync.dma_start(ht_sb[:], ht_ap)

    # ---- bias[h, b] = sum_k W3T[k,h] * hiddenT[k,b] (all batches at once) ----
    bias_sb = const.tile([128, NM, BPC], F32)  # [p, m, b]
    for m in range(NM):
        bp = bpsum.tile([128, BPC], F32)
        for c in range(2):
            nc.tensor.matmul(bp[:],
                             lhsT=w3_sb[:, c, m * 128:(m + 1) * 128],
                             rhs=ht_sb[:, c, :],
                             start=(c == 0), stop=(c == 1))
        nc.vector.tensor_copy(bias_sb[:, m, :], bp[:])

    for b in range(BPC):
        # stream this batch's data: 4 k-chunks of [128, S]
        xs = []
        for src, base in ((static_ap, 0), (dyn_ap, 2)):
            for c in range(2):
                xt = xpool.tile([128, S], F32R, tag="x")
                nc.sync.dma_start(xt[:], src[b, c * 128:(c + 1) * 128, :])
                xs.append(xt)

        exp_row = rows.tile([1, S], F32, tag="exp")
        sums = tiny.tile([1, NS], F32, tag="sums")

        for sg in range(NS // 2):  # s-tile pairs: one LDWEIGHTS per 2 matmuls
            es_tiles = []
            for m in range(NM):
                eps = [epsum.tile([128, ST], F32, tag="ep", name=f"ep{j}") for j in range(2)]
                for c in range(NK):
                    for j in range(2):
                        s = 2 * sg + j
                        nc.tensor.matmul(eps[j][:],
                                         lhsT=wt_sb[:, c, m * 128:(m + 1) * 128],
                                         rhs=xs[c][:, s * ST:(s + 1) * ST],
                                         start=(c == 0), stop=(c == NK - 1))
                row = []
                for j in range(2):
                    es = esb.tile([128, ST], F32R, tag="es")
                    nc.scalar.activation(es[:], eps[j][:],
                                         TANH, bias=bias_sb[:, m, b:b + 1])
                    row.append(es)
                es_tiles.append(row)

            for j in range(2):
                s = 2 * sg + j
                sp = spsum.tile([1, ST], F32)
                nc.tensor.matmul(sp[:], lhsT=vt_sb[:, 0:1], rhs=es_tiles[0][j][:],
                                 start=True, stop=False)
                nc.tensor.matmul(sp[:], lhsT=vt_sb[:, 1:2], rhs=es_tiles[1][j][:],
                                 start=False, stop=True)
                nc.scalar.activation(exp_row[:, s * ST:(s + 1) * ST], sp[:],
                                     EXP, accum_out=sums[:, s:s + 1])

        tot = tiny.tile([1, 1], F32, tag="tot")
        nc.vector.tensor_reduce(tot[:], sums[:], axis=mybir.AxisListType.X,
                                op=mybir.AluOpType.add)
        inv = tiny.tile([1, 1], F32, tag="inv")
        nc.vector.reciprocal(inv[:], tot[:])
        nc.vector.tensor_scalar_mul(exp_row[:], exp_row[:], inv[:, 0:1])
        nc.sync.dma_start(out_ap[b:b + 1, :], exp_row[:])


_CACHED = None


def _build():
    global _CACHED
    if _CACHED is not None:
        return _CACHED
    nc = bacc.Bacc("TRN2", target_bir_lowering=False, debug=False,
                   num_devices=NCORES)
    static = nc.dram_tensor("static", (BPC, H, S), F32R, kind="ExternalInput").ap()
    dyn = nc.dram_tensor("dynamic", (BPC, H, S), F32R, kind="ExternalInput").ap()
    wt = nc.dram_tensor("wt", (128, NK, H), F32R, kind="ExternalInput").ap()
    w3t = nc.dram_tensor("w3t", (128, 2, H), F32R, kind="ExternalInput").ap()
    vt = nc.dram_tensor("vt", (128, 2), F32R, kind="ExternalInput").ap()
    ht = nc.dram_tensor("ht", (128, 2, BPC), F32R, kind="ExternalInput").ap()
    out = nc.dram_tensor("out", (BPC, S), F32, kind="ExternalOutput").ap()

    with tile.TileContext(nc) as tc:
        _attn_kernel(tc, out, static, dyn, wt, w3t, vt, ht)
    nc.compile()
    _CACHED = nc
    return nc


def _chunk_major(a: np.ndarray) -> np.ndarray:
    """[C*128, F] -> [128, C, F] so partition p holds rows {p, 128+p, ...}."""
    c = a.shape[0] // 128
    return np.ascontiguousarray(a.reshape(c, 128, -1).transpose(1, 0, 2))


def kernel(static_enc, dynamic_enc, decoder_hidden, v, W, *, _trace=False,
           **trace_kwargs):
    static_enc = np.ascontiguousarray(static_enc, dtype=np.float32)
    dynamic_enc = np.ascontiguousarray(dynamic_enc, dtype=np.float32)
    decoder_hidden = np.ascontiguousarray(decoder_hidden, dtype=np.float32)
    v = np.ascontiguousarray(v, dtype=np.float32)
    W = np.ascontiguousarray(W, dtype=np.float32)

    nc = _build()

    wt = _chunk_major(np.concatenate([W[:, :H].T, W[:, H:2 * H].T], axis=0))
    w3t = _chunk_major(np.ascontiguousarray(W[:, 2 * H:].T))
    vt = np.ascontiguousarray(v.reshape(2, 128).T)          # [128, 2]
    in_maps = []
    for i in range(NCORES):
        sl = slice(i * BPC, (i + 1) * BPC)
        ht = _chunk_major(np.ascontiguousarray(decoder_hidden[sl].T))
        in_maps.append({
            "static": static_enc[sl],
            "dynamic": dynamic_enc[sl],
            "wt": wt, "w3t": w3t, "vt": vt, "ht": ht,
        })

    res = run_bass_kernel_spmd(nc, in_maps, core_ids=list(range(NCORES)),
                               trace=_trace, **trace_kwargs)
    kernel.last_result = res
    return np.concatenate([res.results[i]["out"] for i in range(NCORES)], axis=0)


kernel.last_result = None
